# revision 1
# baseline (speedup 1.0000x reference)
"""Trainium2 Bass kernel for a diagonal-A linear dynamical system (LDS).

    Bu = inputs @ B            [B, T, S]
    h_t = h_{t-1} * A + Bu_t   (scan over T, diagonal A)
    y_t = h_t @ C              [B, T, O]

Shapes: inputs [16, 4096, 256], A [256], B [256, 256], C [256, 256],
h0 [256]; all float32.

Sharding: data-parallel over batch across 8 NeuronCores (2 batches per
core); A/B/C/h0 replicated.

v4 design: R=2 pair-step scan decomposition. The DVE scan is the
critical resource (TensorTensorScanArith runs ~2 cyc/column on DVE and
no other engine supports it), so we halve its column count by scanning
only odd timesteps:

    h_{2u+1} = A^2 * h_{2u-1} + v_u,   v_u = u_{2u} @ B' + u_{2u+1} @ B
    y_{2u+1} = h_{2u+1} @ C
    y_{2u}   = h_{2u-1} @ C' + u_{2u} @ BC

with B' = B diag(A), C' = diag(A) C, BC = B @ C all precomputed on the
host (B, C scaled by 16 so BC stays in fp16 normal range; the ACT
PSUM->SBUF copy multiplies y by 1/256 to undo it). PE work grows 25%
(the u_even @ BC path) but DVE halves.

Layouts: host pre-transposes u to [i, t] with even/odd t split per
supertile; y returns as [o, t] even/odd split and the host
re-interleaves. fp16 data path everywhere (PSUM accumulation and scan
state stay fp32).
"""

import numpy as np

import concourse.bacc as bacc
import concourse.bass as bass
import concourse.mybir as mybir
import concourse.tile as tile
from concourse import bass_utils

BATCH, T, D = 16, 4096, 256
NCORES = 8
BLOC = BATCH // NCORES  # batches per core
TT = 1024               # time supertile == chunk (512 even + 512 odd)
NJ = T // TT            # supertiles per sequence
SC = TT // 2            # scan columns / matmul moving width per chunk
HL = T // 2 + 1         # odd-state buffer length (incl. h0 guard col)
WSCALE = 16.0           # host scales B, C by this; y copy undoes ^2
F32 = mybir.dt.float32
F16 = mybir.dt.float16

_CACHE: dict = {}


def _build_nc():
    nc = bacc.Bacc(trn_type="TRN2", target_bir_lowering=False)

    # u[b, k, p, j, par, e] = inputs[b, j*TT + 2*e + par, k*128+p]
    u = nc.dram_tensor("u", [BLOC, 2, 128, T], F16, kind="ExternalInput")
    # W1[p, w, k, d]: w=0 Bp=B*diag(A), w=1 B   (i = k*128+p)
    W1d = nc.dram_tensor("W1", [128, 2, 2, D], F16, kind="ExternalInput")
    # W2[p, w, k, d]: w=0 C, w=1 Cp=diag(A)*C, w=2 BC=B@C
    W2d = nc.dram_tensor("W2", [128, 3, 2, D], F16, kind="ExternalInput")
    # S[p, c]: c=0,1 A^2 halves; c=2,3 h0 halves
    Sd = nc.dram_tensor("S", [128, 4], F32, kind="ExternalInput")
    # y[b, m, p, j, par, e] = out[b, j*TT + 2*e + par, m*128+p] * WSCALE^2
    y = nc.dram_tensor("y", [BLOC, 2, 128, T], F16, kind="ExternalOutput")

    u_r = u[:].rearrange("b k p (j par e) -> b j p k par e", par=2, e=SC)
    y_r = y[:].rearrange("b m p (j par e) -> b j p m par e", par=2, e=SC)

    mult = mybir.AluOpType.mult
    add = mybir.AluOpType.add

    with tile.TileContext(nc) as tc:
        with (
            tc.tile_pool(name="const", bufs=1) as const,
            tc.tile_pool(name="upool", bufs=6) as upool,
            tc.tile_pool(name="ypool", bufs=2) as ypool,
            tc.tile_pool(name="hpool", bufs=1) as hpool,
            tc.tile_pool(name="ps_v", bufs=4, space="PSUM") as ps_v,
            tc.tile_pool(name="ps_y", bufs=2, space="PSUM") as ps_y,
        ):
            u_t: dict = {}

            def dma_u(b, j):
                u_t[(b, j)] = upool.tile(
                    [128, 2, 2, SC], F16, tag="u_t", name="u_t"
                )
                nc.sync.dma_start(u_t[(b, j)], u_r[b, j])

            # --- DMA issue order is critical: each dma_start costs ~0.65us
            # of serial issue time on its sequencer. The first u tile gates
            # the first matmul, so it goes FIRST on the Sync hw-DGE rail
            # with nothing queued ahead; the consts issue concurrently on
            # the Activation hw-DGE rail (small, land early regardless). ---
            for b in range(BLOC):
                dma_u(b, 0)
            W1 = const.tile([128, 2, 2, D], F16, name="W1")
            nc.scalar.dma_start(W1, W1d[:])
            W2 = const.tile([128, 3, 2, D], F16, name="W2")
            nc.scalar.dma_start(W2, W2d[:])
            Sc = const.tile([128, 4], F32, name="Sc")
            nc.scalar.dma_start(Sc, Sd[:])

            Bp_sb, B_sb = W1[:, 0], W1[:, 1]          # [128, 2, D]
            C_sb, Cp_sb, BC_sb = W2[:, 0], W2[:, 1], W2[:, 2]
            A2_col, h0c = Sc[:, 0:2], Sc[:, 2:4]

            ones = const.tile([128, SC], F32, name="ones")
            nc.vector.memset(ones, 1.0)
            A2_bc = const.tile([128, 2, SC], F32, name="A2_bc")
            for m in range(2):
                nc.scalar.mul(A2_bc[:, m], ones, mul=A2_col[:, m : m + 1])

            # odd hidden states h_1, h_3, ...; col 0 is the h0 guard
            hT = hpool.tile([128, BLOC, 2, HL], F16, name="hT")
            for b in range(BLOC):
                for m in range(2):
                    nc.scalar.copy(hT[:, b, m, 0:1], h0c[:, m : m + 1])

            def emit_v_scan(b, j):
                ut = u_t[(b, j)]
                vs = []
                for m in range(2):
                    ms = slice(m * 128, (m + 1) * 128)
                    v = ps_v.tile([128, SC], F32, tag="v", name="v")
                    vs.append(v)
                    for k in range(2):
                        nc.tensor.matmul(
                            v, Bp_sb[:, k, ms], ut[:, k, 0],
                            start=(k == 0), stop=False,
                        )
                    for k in range(2):
                        nc.tensor.matmul(
                            v, B_sb[:, k, ms], ut[:, k, 1],
                            start=False, stop=(k == 1),
                        )
                for m in range(2):
                    init = (
                        h0c[:, m : m + 1]
                        if j == 0
                        else hT[:, b, m, j * SC : j * SC + 1]
                    )
                    nc.vector.tensor_tensor_scan(
                        hT[:, b, m, 1 + j * SC : 1 + (j + 1) * SC],
                        A2_bc[:, m],
                        vs[m],
                        init,
                        op0=mult,
                        op1=add,
                    )

            def emit_y(b, j):
                inv = 1.0 / (WSCALE * WSCALE)
                ysb = ypool.tile([128, 2, 2, SC], F16, tag="y_sb", name="y_sb")
                for m in range(2):
                    ms = slice(m * 128, (m + 1) * 128)
                    yod = ps_y.tile([128, SC], F32, tag="yod", name="yod")
                    yev = ps_y.tile([128, SC], F32, tag="yev", name="yev")
                    for k in range(2):
                        nc.tensor.matmul(
                            yod, C_sb[:, k, ms],
                            hT[:, b, k, 1 + j * SC : 1 + (j + 1) * SC],
                            start=(k == 0), stop=(k == 1),
                        )
                    for k in range(2):
                        nc.tensor.matmul(
                            yev, Cp_sb[:, k, ms],
                            hT[:, b, k, j * SC : (j + 1) * SC],
                            start=(k == 0), stop=False,
                        )
                    for k in range(2):
                        nc.tensor.matmul(
                            yev, BC_sb[:, k, ms],
                            u_t[(b, j)][:, k, 0],
                            start=False, stop=(k == 1),
                        )
                    nc.scalar.mul(ysb[:, m, 1, :], yod, mul=inv)
                    nc.scalar.mul(ysb[:, m, 0, :], yev, mul=inv)
                nc.sync.dma_start(y_r[b, j], ysb)

            for j in range(NJ):
                if j + 1 < NJ:
                    for b in range(BLOC):
                        dma_u(b, j + 1)
                for b in range(BLOC):
                    emit_v_scan(b, j)
                if j >= 1:
                    for b in range(BLOC):
                        emit_y(b, j - 1)
            for b in range(BLOC):
                emit_y(b, NJ - 1)

    nc.compile()
    return nc


def _get_nc():
    if "nc" not in _CACHE:
        _CACHE["nc"] = _build_nc()
    return _CACHE["nc"]


def make_in_maps(inputs, A, B, C, h0):
    u = np.asarray(inputs, dtype=np.float32)
    # [B, T, 2, 128] -> [B, 2, 128, T] -> even/odd split per supertile
    uT = u.reshape(BATCH, T, 2, 128).transpose(0, 2, 3, 1)
    u4 = uT.reshape(BATCH, 2, 128, NJ, SC, 2).transpose(0, 1, 2, 3, 5, 4)
    uT = np.ascontiguousarray(u4).reshape(BATCH, 2, 128, T).astype(np.float16)

    Af = np.asarray(A, np.float32)
    Bf = np.asarray(B, np.float32) * WSCALE
    Cf = np.asarray(C, np.float32) * WSCALE
    Bp = Bf * Af[None, :]          # B * diag(A)
    Cp = Cf * Af[:, None]          # diag(A) * C
    BC = Bf @ Cf                   # (16B) @ (16C) = 256 * B@C

    def wsplit(M):  # [256, 256] -> [128, 2, 256] (p, k, d)
        return M.reshape(2, 128, D).transpose(1, 0, 2)

    # W1[p, w, k, d], W2[p, w, k, d]
    W1 = np.ascontiguousarray(
        np.stack([wsplit(Bp), wsplit(Bf)], axis=1)
    ).astype(np.float16)
    W2 = np.ascontiguousarray(
        np.stack([wsplit(Cf), wsplit(Cp), wsplit(BC)], axis=1)
    ).astype(np.float16)
    A2 = (Af * Af).reshape(2, 128).T
    h02 = (np.asarray(h0, np.float32) * WSCALE).reshape(2, 128).T
    S = np.ascontiguousarray(np.concatenate([A2, h02], axis=1), dtype=np.float32)
    core_consts = {"W1": W1, "W2": W2, "S": S}
    return [
        {"u": np.ascontiguousarray(uT[c * BLOC : (c + 1) * BLOC]), **core_consts}
        for c in range(NCORES)
    ]


def kernel(inputs, A, B, C, h0, _trace=False):
    nc = _get_nc()
    in_maps = make_in_maps(inputs, A, B, C, h0)
    res = bass_utils.run_bass_kernel_spmd(
        nc, in_maps, core_ids=list(range(NCORES)), trace=_trace
    )
    outs = []
    for r in res.results:
        yT = r["y"].astype(np.float32)  # [BLOC, 2, 128, T], even/odd split
        y4 = yT.reshape(BLOC, 2, 128, NJ, 2, SC).transpose(0, 1, 2, 3, 5, 4)
        yT = y4.reshape(BLOC, 2, 128, T)  # natural t order
        outs.append(np.moveaxis(yT, 3, 1).reshape(BLOC, T, D))
    out = np.concatenate(outs, axis=0)
    if _trace:
        _CACHE["last_result"] = res
    return out

